# revision 18
# baseline (speedup 1.0000x reference)
"""Trainium2 Bass kernel for DigitConvolutionalModel:
    out = relu(conv2d_3x3_valid(x.reshape(B,28,28))) .reshape(B,676) @ W + b

Strategy (pure data parallel over 8 cores, B=32768 -> 4096/core):

Per core, samples are processed in 8 supergroups (SG) of 512 = 128
"quads" of 4 consecutive samples. Host marshals x to a padded
128-partition layout per SG (4 sample-slots x 32 rows, rows 28-31
zero) so every SDMA port participates in the load:
    X[32k + r, (c, bq)] = x[sample(k,bq,s), 28r + c],  r<28
All 8 SG loads are issued up-front on both HWDGE rings (SG0 split in
two column-halves so the first conv matmul starts ~1.8us in).

Conv (cross-correlation) is 3 PSUM-accumulated full-array matmuls per
column-chunk, contracting all 128 rows at once with a block-diagonal
Toeplitz lhsT (M=128, blocks of 26 output rows padded to 32):
    msb[32k + i + di, 128dj + 32k + i] = conv_w[di, dj]
    Y[32k + i, (c, bq)] = sum_dj msb_dj^T @ X[:, (c + dj, bq)]
ReLU copies PSUM->SBUF (alternating DVE/ACT) into fp16
h_all[32k + i, s*3328 + c*128 + bq].

The FC contracts i for all 4 quad-slots at once with a block-diagonal
W lhsT (K=128, M=128), one matmul per output column c over FOUR
supergroups at once (N=512, rhs strided over the s axis of h_all):
    outT[32k + o, (s, bq)] += wsb_c^T @ h_all[:, (s, c, bq)]
so the 26 weight loads are paid twice per pass instead of 8 times.
Bias is added on DVE with a per-partition scalar; DVE 32x32 block
transposes flip outT into sample-major order for a contiguous 40B-run
store (single DMA at the end, on the then-idle SP ring).

All matmul operands are fp16 (values are O(10), well in range); PSUM
accumulates fp32, so the only precision loss is ~2^-11 input rounding.
"""

import sys
import numpy as np

for _p in ("/opt/trn_rl_repo", "/root/.axon_site/_ro/trn_rl_repo"):
    if _p not in sys.path:
        sys.path.insert(0, _p)

import concourse.bass as bass  # noqa: E402,F401
import concourse.tile as tile  # noqa: E402
from concourse import bacc, mybir  # noqa: E402
from concourse.bass_utils import run_bass_kernel_spmd  # noqa: E402

IMG = 28
KW = 3
OUT = 26  # IMG - KW + 1
NPIX = IMG * IMG          # 784
NOUTPIX = OUT * OUT       # 676
NCLS = 10
NCORES = 8
B_TOTAL = 32768
B_CORE = B_TOTAL // NCORES   # 4096
SG = 512                     # samples per supergroup (128 quads of 4)
N_SG = B_CORE // SG          # 8
NQ = 7                       # column chunks: 6x4 + 1x2 = 26 columns
HSTRIDE = OUT * 128          # 3328: per-SG h stride in h_all
F32 = mybir.dt.float32
F16 = mybir.dt.float16

_CACHE = {}


def _chunk_cols(q):
    """(first output column, n columns) of chunk q."""
    return 4 * q, (2 if q == NQ - 1 else 4)


def _build_program(mm_dtype=F16, n_sg=N_SG, hwloop=0, stage=5):
    """Build + compile the per-core Bass program (identical on all cores)."""
    nc = bacc.Bacc("TRN2", target_bir_lowering=False, debug=False,
                   num_devices=NCORES)

    x_d = nc.dram_tensor("x", (N_SG, 128, 128 * IMG), mm_dtype,
                         kind="ExternalInput")
    msb_d = nc.dram_tensor("msb", (128, 3 * 128), mm_dtype,
                           kind="ExternalInput")
    wsb_d = nc.dram_tensor("wsb", (128, OUT * 128), mm_dtype,
                           kind="ExternalInput")
    bias_d = nc.dram_tensor("biasv", (128, 1), F32, kind="ExternalInput")
    out_d = nc.dram_tensor("out", (B_CORE, NCLS), F32, kind="ExternalOutput")

    x_ap = x_d.ap()
    out_ap = out_d.ap()
    NFC = 4                      # supergroups per FC batch
    SPLIT0 = 16                  # SG0 half A columns [0,16), B [12,28)

    with tile.TileContext(nc) as tc:
        with (
            tc.tile_pool(name="consts", bufs=1) as consts,
            tc.tile_pool(name="xin", bufs=1) as xin,
            tc.tile_pool(name="hbuf", bufs=1) as hbuf,
            tc.tile_pool(name="obuf", bufs=2) as obuf,
            tc.tile_pool(name="convps", bufs=6, space="PSUM") as convps,
            tc.tile_pool(name="fcps", bufs=2, space="PSUM") as fcps,
        ):
            msb = consts.tile([128, 3 * 128], mm_dtype)
            wsb = consts.tile([128, OUT * 128], mm_dtype)
            biasv = consts.tile([128, 1], F32)
            nc.sync.dma_start(out=msb[:, :], in_=msb_d.ap())
            nc.sync.dma_start(out=wsb[:, :], in_=wsb_d.ap())
            nc.sync.dma_start(out=biasv[:, :], in_=bias_d.ap())

            import contextlib
            loop_cm = (tc.For_i(0, hwloop, 1) if hwloop
                       else contextlib.nullcontext())
            with loop_cm:
                # ---- all SG loads up-front, spread over both HWDGE rings;
                # SG0 in two column-halves so conv can start early ----
                xt0a = xin.tile([128, SPLIT0 * 128], mm_dtype, tag="xt0a")
                xt0b = xin.tile([128, SPLIT0 * 128], mm_dtype, tag="xt0b")
                x3 = x_ap.rearrange("s p (c b) -> s p c b", b=128)
                nc.sync.dma_start(
                    out=xt0a[:, :].rearrange("p (c b) -> p c b", b=128),
                    in_=x3[0, :, 0:SPLIT0, :])
                nc.scalar.dma_start(
                    out=xt0b[:, :].rearrange("p (c b) -> p c b", b=128),
                    in_=x3[0, :, IMG - SPLIT0:IMG, :])
                xts = [None]
                for s in range(1, n_sg):
                    xt = xin.tile([128, 128 * IMG], mm_dtype, tag=f"xt{s}")
                    eng = nc.sync if s % 2 == 1 else nc.scalar
                    eng.dma_start(out=xt[0:128, :], in_=x_ap[s])
                    xts.append(xt)

                h_all = hbuf.tile([128, n_sg * HSTRIDE], mm_dtype, tag="h")
                hv = h_all[:, :].rearrange("p (s c b) -> p s c b",
                                           s=n_sg, b=128)
                outsb = obuf.tile([128, n_sg * 4 * NCLS], F32, tag="outsb")

                def fc_batch(half):
                    """FC + bias + transpose for SGs [half*NFC, +NFC)."""
                    s0 = half * NFC
                    ot = fcps.tile([128, 512], F32, tag="ot")
                    for c in range(OUT):
                        nc.tensor.matmul(
                            ot[0:128, 0:512],
                            wsb[0:128, 128 * c:128 * c + 128],
                            hv[:, s0:s0 + NFC, c, :],
                            start=(c == 0), stop=(c == OUT - 1),
                        )
                    # bias add (per-partition scalar) PSUM -> SBUF
                    osb = obuf.tile([128, 512], F32, tag="osb")
                    nc.vector.tensor_scalar_add(osb[:, :], ot[:, :],
                                                biasv[:, 0:1])
                    if stage < 5:
                        return
                    # DVE 32x32 block transpose: osb[32k+o, 128sl+bq] ->
                    # tt[32k + bq%32, 32*(4sl + bq//32) + o]
                    tt = obuf.tile([128, 512], F32, tag="ttbuf")
                    for k in range(4):
                        nc.vector.transpose(tt[32 * k:32 * k + 32, 0:512],
                                            osb[32 * k:32 * k + 32, 0:512])
                    # gather valid o<10 cols of each 32-block into
                    # outsb[:, (4half + sl)*40 + hi*10 + o]
                    nc.vector.tensor_copy(
                        outsb[:, half * NFC * 40:(half + 1) * NFC * 40]
                        .rearrange("p (sl hi o) -> p sl hi o", sl=4, hi=4),
                        tt[:, :].rearrange("p (sl hi o) -> p sl hi o",
                                           sl=4, hi=4)[:, :, :, 0:NCLS])
                    if half == n_sg // NFC - 1:
                        # sample n = (32k + bq%32)*32 + 4s + bq//32:
                        # one DMA, 40B contiguous runs; SP ring is idle now
                        nc.sync.dma_start(
                            out=out_ap[:, :].rearrange(
                                "(p ss hi) o -> p ss hi o", ss=8, hi=4),
                            in_=outsb[:, :].rearrange(
                                "p (ss hi o) -> p ss hi o", ss=8, hi=4))

                for s in range(n_sg):
                    if stage < 2:
                        break
                    # ---- conv: per column-chunk q, 3 accumulated matmuls
                    for q in range(NQ):
                        c0, ncol = _chunk_cols(q)
                        pq = convps.tile([128, 512], F32, tag="pq")
                        for dj in range(3):
                            if s == 0:
                                if q < 3:
                                    xv = xt0a[:, :].rearrange(
                                        "p (c b) -> p c b", b=128)
                                    cl = c0 + dj
                                else:
                                    xv = xt0b[:, :].rearrange(
                                        "p (c b) -> p c b", b=128)
                                    cl = c0 + dj - (IMG - SPLIT0)
                            else:
                                xv = xts[s][:, :].rearrange(
                                    "p (c b) -> p c b", b=128)
                                cl = c0 + dj
                            nc.tensor.matmul(
                                pq[0:128, 0:ncol * 128],
                                msb[0:128, 128 * dj:128 * dj + 128],
                                xv[:, cl:cl + ncol, :],
                                start=(dj == 0), stop=(dj == 2),
                            )
                        # ---- relu PSUM -> SBUF (c-major within SG s) ----
                        if stage < 3:
                            continue
                        hslice = h_all[:, s * HSTRIDE + c0 * 128:
                                       s * HSTRIDE + (c0 + ncol) * 128]
                        if (s * NQ + q) % 2 == 0:
                            nc.vector.tensor_scalar_max(
                                hslice, pq[:, 0:ncol * 128], 0.0)
                        else:
                            nc.scalar.activation(
                                hslice, pq[:, 0:ncol * 128],
                                mybir.ActivationFunctionType.Relu)

                    # ---- FC for 4 SGs, right after their conv ----
                    if stage >= 4 and s % NFC == NFC - 1:
                        fc_batch(s // NFC)

                if stage < 2:
                    # DMA-only ablation: consume one tile trivially
                    dmy = obuf.tile([128, 8], F32, tag="dmy")
                    nc.vector.tensor_copy(dmy[0:1, 0:8],
                                          xts[1][0:1, 0:16].bitcast(F32))

    nc.compile()
    return nc


def _host_constants(conv_w, W, b):
    """Block-diagonal Toeplitz conv lhsT and block-diagonal FC lhsT."""
    msb = np.zeros((128, 3 * 128), np.float32)
    for dj in range(KW):
        for k in range(4):
            for i in range(OUT):
                for di in range(KW):
                    msb[32 * k + i + di, 128 * dj + 32 * k + i] = conv_w[di, dj]
    wsb = np.zeros((128, OUT * 128), np.float32)
    for c in range(OUT):
        for k in range(4):
            for i in range(OUT):
                wsb[32 * k + i, 128 * c + 32 * k:128 * c + 32 * k + NCLS] = \
                    W[i * OUT + c, :]
    biasv = np.zeros((128, 1), np.float32)
    for k in range(4):
        biasv[32 * k:32 * k + NCLS, 0] = b
    return msb, wsb, biasv


def _marshal_x(x):
    """[B, 784] fp32 -> per-core [N_SG, 128, 3584] fp16 padded layout."""
    # sample n = (32k + bq%32)*32 + 4s + bq//32; bq = 32*hi + bl
    # n axes: [k(4), bl(32), s(8), hi(4)]
    xs = x.reshape(NCORES, 4, 32, N_SG, 4, IMG, IMG)  # core k bl s hi r c
    xs = xs.transpose(0, 3, 1, 5, 6, 4, 2)            # core s k r c hi bl
    xp = np.zeros((NCORES, N_SG, 4, 32, IMG, 4, 32), np.float16)
    xp[:, :, :, 0:IMG] = xs
    return xp.reshape(NCORES, N_SG, 128, 128 * IMG)


def _run(x, conv_w, W, b, trace=False, mm_dtype=F16):
    x = np.ascontiguousarray(np.asarray(x, dtype=np.float32))
    conv_w = np.asarray(conv_w, dtype=np.float32)
    W = np.asarray(W, dtype=np.float32)
    b = np.asarray(b, dtype=np.float32)
    assert x.shape == (B_TOTAL, NPIX), x.shape

    key = ("prog", str(mm_dtype))
    if key not in _CACHE:
        _CACHE[key] = _build_program(mm_dtype)
    nc = _CACHE[key]

    msb, wsb, biasv = _host_constants(conv_w, W, b)
    msb_r, wsb_r = msb.astype(np.float16), wsb.astype(np.float16)
    xm = _marshal_x(x)
    in_maps = []
    for i in range(NCORES):
        in_maps.append({
            "x": xm[i],
            "msb": msb_r, "wsb": wsb_r, "biasv": biasv,
        })
    res = run_bass_kernel_spmd(nc, in_maps, core_ids=list(range(NCORES)),
                               trace=trace)
    out = np.concatenate([res.results[i]["out"] for i in range(NCORES)],
                         axis=0)
    return out, res


def kernel(x, conv_w, W, b):
    out, _ = _run(x, conv_w, W, b, trace=False)
    return out


# revision 27
# speedup vs baseline: 1.5753x; 1.5753x over previous
"""Trainium2 Bass kernel for DigitConvolutionalModel:
    out = relu(conv2d_3x3_valid(x.reshape(B,28,28))) .reshape(B,676) @ W + b

Strategy (pure data parallel over 8 cores, B=32768 -> 4096/core):

Per core, samples are processed in 8 supergroups (SG) of 512 = 128
"quads" of 4 consecutive samples. Host marshals x to a padded
128-partition layout per SG (4 sample-slots x 32 rows, rows 28-31
zero) so every SDMA port participates in the load:
    X[32k + r, (c, bq)] = x[sample(k,bq,s), 28r + c],  r<28
All 8 SG loads are issued up-front on both HWDGE rings (SG0 split in
two column-halves so the first conv matmul starts ~1.8us in).

Conv (cross-correlation) is 3 PSUM-accumulated full-array matmuls per
column-chunk, contracting all 128 rows at once with a block-diagonal
Toeplitz lhsT (M=128, blocks of 26 output rows padded to 32):
    msb[32k + i + di, 128dj + 32k + i] = conv_w[di, dj]
    Y[32k + i, (c, bq)] = sum_dj msb_dj^T @ X[:, (c + dj, bq)]
ReLU copies PSUM->SBUF (alternating DVE/ACT) into fp16
h_all[32k + i, s*3328 + c*128 + bq].

The FC contracts i for all 4 quad-slots at once with a block-diagonal
W lhsT (K=128, M=128), one matmul per output column c over FOUR
supergroups at once (N=512, rhs strided over the s axis of h_all):
    outT[32k + o, (s, bq)] += wsb_c^T @ h_all[:, (s, c, bq)]
so the 26 weight loads are paid twice per pass instead of 8 times.
Bias is added on DVE with a per-partition scalar; DVE 32x32 block
transposes flip outT into sample-major order for a contiguous 40B-run
store (single DMA at the end, on the then-idle SP ring).

All matmul operands are fp16 (values are O(10), well in range); PSUM
accumulates fp32, so the only precision loss is ~2^-11 input rounding.
"""

import sys
import numpy as np

for _p in ("/opt/trn_rl_repo", "/root/.axon_site/_ro/trn_rl_repo"):
    if _p not in sys.path:
        sys.path.insert(0, _p)

import concourse.bass as bass  # noqa: E402,F401
import concourse.tile as tile  # noqa: E402
from concourse import bacc, mybir  # noqa: E402
from concourse.bass_utils import run_bass_kernel_spmd  # noqa: E402

IMG = 28
KW = 3
OUT = 26  # IMG - KW + 1
NPIX = IMG * IMG          # 784
NOUTPIX = OUT * OUT       # 676
NCLS = 10
NCORES = 8
B_TOTAL = 32768
B_CORE = B_TOTAL // NCORES   # 4096
SG = 512                     # samples per supergroup (128 quads of 4)
N_SG = B_CORE // SG          # 8
NQ = 7                       # column chunks: 6x4 + 1x2 = 26 columns
HSTRIDE = OUT * 128          # 3328: per-SG h stride in h_all
F32 = mybir.dt.float32
F16 = mybir.dt.float16

_CACHE = {}


def _chunk_cols(q):
    """(first output column, n columns) of chunk q."""
    return 4 * q, (2 if q == NQ - 1 else 4)


def _build_program(mm_dtype=F16, n_sg=N_SG, hwloop=0, stage=5):
    """Build + compile the per-core Bass program (identical on all cores)."""
    nc = bacc.Bacc("TRN2", target_bir_lowering=False, debug=False,
                   num_devices=NCORES)

    x_d = nc.dram_tensor("x", (N_SG, 128, 128 * IMG), mm_dtype,
                         kind="ExternalInput")
    msb_d = nc.dram_tensor("msb", (128, 3 * 128), mm_dtype,
                           kind="ExternalInput")
    wsb_d = nc.dram_tensor("wsb", (128, OUT * 128), mm_dtype,
                           kind="ExternalInput")
    bias_d = nc.dram_tensor("biasv", (128, 1), F32, kind="ExternalInput")
    out_d = nc.dram_tensor("out", (B_CORE, NCLS), F32, kind="ExternalOutput")

    x_ap = x_d.ap()
    out_ap = out_d.ap()
    NFC = 4                      # supergroups per FC batch
    SPLIT0 = 16                  # SG0 half A columns [0,16), B [12,28)

    with tile.TileContext(nc) as tc:
        with (
            tc.tile_pool(name="consts", bufs=1) as consts,
            tc.tile_pool(name="xin", bufs=1) as xin,
            tc.tile_pool(name="hbuf", bufs=1) as hbuf,
            tc.tile_pool(name="obuf", bufs=2) as obuf,
            tc.tile_pool(name="convps", bufs=7, space="PSUM") as convps,
            tc.tile_pool(name="fcps", bufs=1, space="PSUM") as fcps,
        ):
            msb = consts.tile([128, 3 * 128], mm_dtype)
            wsb = consts.tile([128, OUT * 128], mm_dtype)
            biasv = consts.tile([128, 1], F32)
            nc.sync.dma_start(out=msb[:, :], in_=msb_d.ap())
            nc.sync.dma_start(out=wsb[:, :], in_=wsb_d.ap())
            nc.sync.dma_start(out=biasv[:, :], in_=bias_d.ap())

            import contextlib
            loop_cm = (tc.For_i(0, hwloop, 1) if hwloop
                       else contextlib.nullcontext())
            with loop_cm:
                # ---- all SG loads up-front, spread over both HWDGE rings;
                # SG0 in two column-halves so conv can start early ----
                xt0a = xin.tile([128, SPLIT0 * 128], mm_dtype, tag="xt0a")
                xt0b = xin.tile([128, SPLIT0 * 128], mm_dtype, tag="xt0b")
                x3 = x_ap.rearrange("s p (c b) -> s p c b", b=128)
                nc.sync.dma_start(
                    out=xt0a[:, :].rearrange("p (c b) -> p c b", b=128),
                    in_=x3[0, :, 0:SPLIT0, :])
                nc.scalar.dma_start(
                    out=xt0b[:, :].rearrange("p (c b) -> p c b", b=128),
                    in_=x3[0, :, IMG - SPLIT0:IMG, :])
                xts = [None]
                for s in range(1, n_sg):
                    xt = xin.tile([128, 128 * IMG], mm_dtype, tag=f"xt{s}")
                    eng = nc.sync if s % 2 == 1 else nc.scalar
                    eng.dma_start(out=xt[0:128, :], in_=x_ap[s])
                    xts.append(xt)

                # prime the PE p-state ramp during the xt0a load wait:
                # ~3us of dummy matmuls on already-loaded constants
                prime = fcps.tile([128, 512], F32, tag="ot")
                for _ in range(8):
                    nc.tensor.matmul(prime[0:128, 0:512],
                                     wsb[0:128, 0:128],
                                     wsb[0:128, 0:512],
                                     start=True, stop=True)

                h_all = hbuf.tile([128, n_sg * HSTRIDE], mm_dtype, tag="h")
                hv = h_all[:, :].rearrange("p (s c b) -> p s c b",
                                           s=n_sg, b=128)
                outsb = obuf.tile([128, n_sg * 4 * NCLS], F32, tag="outsb")

                def fc_batch(half):
                    """FC + bias + transpose for SGs [half*NFC, +NFC)."""
                    s0 = half * NFC
                    ot = fcps.tile([128, 512], F32, tag="ot")
                    for c in range(OUT):
                        nc.tensor.matmul(
                            ot[0:128, 0:512],
                            wsb[0:128, 128 * c:128 * c + 128],
                            hv[:, s0:s0 + NFC, c, :],
                            start=(c == 0), stop=(c == OUT - 1),
                        )
                    # bias add (per-partition scalar) PSUM -> SBUF
                    osb = obuf.tile([128, 512], F32, tag="osb")
                    nc.vector.tensor_scalar_add(osb[:, :], ot[:, :],
                                                biasv[:, 0:1])
                    if stage < 5:
                        return
                    # DVE 32x32 block transpose: osb[32k+o, 128sl+bq] ->
                    # tt[32k + bq%32, 32*(4sl + bq//32) + o]
                    tt = obuf.tile([128, 512], F32, tag="ttbuf")
                    for k in range(4):
                        nc.vector.transpose(tt[32 * k:32 * k + 32, 0:512],
                                            osb[32 * k:32 * k + 32, 0:512])
                    # gather valid o<10 cols of each 32-block into
                    # outsb[:, (4half + sl)*40 + hi*10 + o]
                    nc.vector.tensor_copy(
                        outsb[:, half * NFC * 40:(half + 1) * NFC * 40]
                        .rearrange("p (sl hi o) -> p sl hi o", sl=4, hi=4),
                        tt[:, :].rearrange("p (sl hi o) -> p sl hi o",
                                           sl=4, hi=4)[:, :, :, 0:NCLS])
                    if half == n_sg // NFC - 1:
                        # sample n = (32k + bq%32)*32 + 4s + bq//32:
                        # one DMA, 40B contiguous runs; SP ring is idle now
                        nc.sync.dma_start(
                            out=out_ap[:, :].rearrange(
                                "(p ss hi) o -> p ss hi o", ss=8, hi=4),
                            in_=outsb[:, :].rearrange(
                                "p (ss hi o) -> p ss hi o", ss=8, hi=4))

                for s in range(n_sg):
                    if stage < 2:
                        break
                    # ---- conv: per column-chunk q, 3 accumulated matmuls
                    for q in range(NQ):
                        c0, ncol = _chunk_cols(q)
                        pq = convps.tile([128, 512], F32, tag="pq")
                        for dj in range(3):
                            if s == 0:
                                if q < 3:
                                    xv = xt0a[:, :].rearrange(
                                        "p (c b) -> p c b", b=128)
                                    cl = c0 + dj
                                else:
                                    xv = xt0b[:, :].rearrange(
                                        "p (c b) -> p c b", b=128)
                                    cl = c0 + dj - (IMG - SPLIT0)
                            else:
                                xv = xts[s][:, :].rearrange(
                                    "p (c b) -> p c b", b=128)
                                cl = c0 + dj
                            nc.tensor.matmul(
                                pq[0:128, 0:ncol * 128],
                                msb[0:128, 128 * dj:128 * dj + 128],
                                xv[:, cl:cl + ncol, :],
                                start=(dj == 0), stop=(dj == 2),
                            )
                        # ---- relu PSUM -> SBUF (c-major within SG s) ----
                        if stage < 3:
                            continue
                        hslice = h_all[:, s * HSTRIDE + c0 * 128:
                                       s * HSTRIDE + (c0 + ncol) * 128]
                        if (s * NQ + q) % 2 == 0:
                            nc.vector.tensor_scalar_max(
                                hslice, pq[:, 0:ncol * 128], 0.0)
                        else:
                            nc.scalar.activation(
                                hslice, pq[:, 0:ncol * 128],
                                mybir.ActivationFunctionType.Relu)

                    # ---- FC for 4 SGs, right after their conv ----
                    if stage >= 4 and s % NFC == NFC - 1:
                        fc_batch(s // NFC)

                if stage < 2:
                    # DMA-only ablation: consume one tile trivially
                    dmy = obuf.tile([128, 8], F32, tag="dmy")
                    nc.vector.tensor_copy(dmy[0:1, 0:8],
                                          xts[1][0:1, 0:16].bitcast(F32))

    nc.compile()
    return nc


def _host_constants(conv_w, W, b):
    """Block-diagonal Toeplitz conv lhsT and block-diagonal FC lhsT."""
    msb = np.zeros((128, 3 * 128), np.float32)
    for dj in range(KW):
        for k in range(4):
            for i in range(OUT):
                for di in range(KW):
                    msb[32 * k + i + di, 128 * dj + 32 * k + i] = conv_w[di, dj]
    wsb = np.zeros((128, OUT * 128), np.float32)
    for c in range(OUT):
        for k in range(4):
            for i in range(OUT):
                wsb[32 * k + i, 128 * c + 32 * k:128 * c + 32 * k + NCLS] = \
                    W[i * OUT + c, :]
    biasv = np.zeros((128, 1), np.float32)
    for k in range(4):
        biasv[32 * k:32 * k + NCLS, 0] = b
    return msb, wsb, biasv


def _marshal_x(x):
    """[B, 784] fp32 -> per-core [N_SG, 128, 3584] fp16 padded layout."""
    # sample n = (32k + bq%32)*32 + 4s + bq//32; bq = 32*hi + bl
    # n axes: [k(4), bl(32), s(8), hi(4)]
    xs = x.reshape(NCORES, 4, 32, N_SG, 4, IMG, IMG)  # core k bl s hi r c
    xs = xs.transpose(0, 3, 1, 5, 6, 4, 2)            # core s k r c hi bl
    xp = np.zeros((NCORES, N_SG, 4, 32, IMG, 4, 32), np.float16)
    xp[:, :, :, 0:IMG] = xs
    return xp.reshape(NCORES, N_SG, 128, 128 * IMG)


def _run(x, conv_w, W, b, trace=False, mm_dtype=F16):
    x = np.ascontiguousarray(np.asarray(x, dtype=np.float32))
    conv_w = np.asarray(conv_w, dtype=np.float32)
    W = np.asarray(W, dtype=np.float32)
    b = np.asarray(b, dtype=np.float32)
    assert x.shape == (B_TOTAL, NPIX), x.shape

    key = ("prog", str(mm_dtype))
    if key not in _CACHE:
        _CACHE[key] = _build_program(mm_dtype)
    nc = _CACHE[key]

    msb, wsb, biasv = _host_constants(conv_w, W, b)
    msb_r, wsb_r = msb.astype(np.float16), wsb.astype(np.float16)
    xm = _marshal_x(x)
    in_maps = []
    for i in range(NCORES):
        in_maps.append({
            "x": xm[i],
            "msb": msb_r, "wsb": wsb_r, "biasv": biasv,
        })
    res = run_bass_kernel_spmd(nc, in_maps, core_ids=list(range(NCORES)),
                               trace=trace)
    out = np.concatenate([res.results[i]["out"] for i in range(NCORES)],
                         axis=0)
    return out, res


def kernel(x, conv_w, W, b):
    out, _ = _run(x, conv_w, W, b, trace=False)
    return out


# revision 31
# speedup vs baseline: 1.6226x; 1.0300x over previous
"""Trainium2 Bass kernel for DigitConvolutionalModel:
    out = relu(conv2d_3x3_valid(x.reshape(B,28,28))) .reshape(B,676) @ W + b

Strategy (pure data parallel over 8 cores, B=32768 -> 4096/core):

Per core, samples are processed in 8 supergroups (SG) of 512 = 128
"quads" of 4 consecutive samples. Host marshals x to a padded
128-partition layout per SG (4 sample-slots x 32 rows, rows 28-31
zero) so every SDMA port participates in the load:
    X[32k + r, (c, bq)] = x[sample(k,bq,s), 28r + c],  r<28
All 8 SG loads are issued up-front on both HWDGE rings (SG0 split in
two column-halves so the first conv matmul starts ~1.8us in).

Conv (cross-correlation) is 3 PSUM-accumulated full-array matmuls per
column-chunk, contracting all 128 rows at once with a block-diagonal
Toeplitz lhsT (M=128, blocks of 26 output rows padded to 32):
    msb[32k + i + di, 128dj + 32k + i] = conv_w[di, dj]
    Y[32k + i, (c, bq)] = sum_dj msb_dj^T @ X[:, (c + dj, bq)]
ReLU copies PSUM->SBUF (alternating DVE/ACT) into fp16
h_all[32k + i, s*3328 + c*128 + bq].

The FC contracts i for all 4 quad-slots at once with a block-diagonal
W lhsT (K=128, M=128), one matmul per output column c over FOUR
supergroups at once (N=512, rhs strided over the s axis of h_all):
    outT[32k + o, (s, bq)] += wsb_c^T @ h_all[:, (s, c, bq)]
so the 26 weight loads are paid twice per pass instead of 8 times.
Bias is added on DVE with a per-partition scalar; DVE 32x32 block
transposes flip outT into sample-major order for a contiguous 40B-run
store (single DMA at the end, on the then-idle SP ring).

All matmul operands are fp16 (values are O(10), well in range); PSUM
accumulates fp32, so the only precision loss is ~2^-11 input rounding.
"""

import sys
import numpy as np

for _p in ("/opt/trn_rl_repo", "/root/.axon_site/_ro/trn_rl_repo"):
    if _p not in sys.path:
        sys.path.insert(0, _p)

import concourse.bass as bass  # noqa: E402,F401
import concourse.tile as tile  # noqa: E402
from concourse import bacc, mybir  # noqa: E402
from concourse.bass_utils import run_bass_kernel_spmd  # noqa: E402

IMG = 28
KW = 3
OUT = 26  # IMG - KW + 1
NPIX = IMG * IMG          # 784
NOUTPIX = OUT * OUT       # 676
NCLS = 10
NCORES = 8
B_TOTAL = 32768
B_CORE = B_TOTAL // NCORES   # 4096
SG = 512                     # samples per supergroup (128 quads of 4)
N_SG = B_CORE // SG          # 8
NQ = 7                       # column chunks: 6x4 + 1x2 = 26 columns
HSTRIDE = OUT * 128          # 3328: per-SG h stride in h_all
F32 = mybir.dt.float32
F16 = mybir.dt.float16

_CACHE = {}


def _chunk_cols(q):
    """(first output column, n columns) of chunk q."""
    return 4 * q, (2 if q == NQ - 1 else 4)


def _build_program(mm_dtype=F16, n_sg=N_SG, hwloop=0, stage=5):
    """Build + compile the per-core Bass program (identical on all cores)."""
    nc = bacc.Bacc("TRN2", target_bir_lowering=False, debug=False,
                   num_devices=NCORES)

    x_d = nc.dram_tensor("x", (N_SG, 128, 128 * IMG), mm_dtype,
                         kind="ExternalInput")
    msb_d = nc.dram_tensor("msb", (128, 3 * 128), mm_dtype,
                           kind="ExternalInput")
    wsb_d = nc.dram_tensor("wsb", (128, OUT * 128), mm_dtype,
                           kind="ExternalInput")
    bias_d = nc.dram_tensor("biasv", (128, 1), F32, kind="ExternalInput")
    out_d = nc.dram_tensor("out", (B_CORE, NCLS), F32, kind="ExternalOutput")

    x_ap = x_d.ap()
    out_ap = out_d.ap()
    NFC = 4                      # supergroups per FC batch
    SPLIT0 = 16                  # SG0 half A columns [0,16), B [12,28)

    with tile.TileContext(nc) as tc:
        with (
            tc.tile_pool(name="consts", bufs=1) as consts,
            tc.tile_pool(name="xin", bufs=1) as xin,
            tc.tile_pool(name="hbuf", bufs=1) as hbuf,
            tc.tile_pool(name="obuf", bufs=2) as obuf,
            tc.tile_pool(name="convps", bufs=7, space="PSUM") as convps,
            tc.tile_pool(name="fcps", bufs=1, space="PSUM") as fcps,
        ):
            msb = consts.tile([128, 3 * 128], mm_dtype)
            wsb = consts.tile([128, OUT * 128], mm_dtype)
            biasv = consts.tile([128, 1], F32)
            nc.sync.dma_start(out=msb[:, :], in_=msb_d.ap())
            nc.sync.dma_start(out=wsb[:, :], in_=wsb_d.ap())
            nc.sync.dma_start(out=biasv[:, :], in_=bias_d.ap())

            import contextlib
            loop_cm = (tc.For_i(0, hwloop, 1) if hwloop
                       else contextlib.nullcontext())
            with loop_cm:
                # ---- all SG loads up-front, spread over both HWDGE rings;
                # SG0 in two column-halves so conv can start early ----
                xt0a = xin.tile([128, SPLIT0 * 128], mm_dtype, tag="xt0a")
                xt0b = xin.tile([128, SPLIT0 * 128], mm_dtype, tag="xt0b")
                x3 = x_ap.rearrange("s p (c b) -> s p c b", b=128)
                nc.sync.dma_start(
                    out=xt0a[:, :].rearrange("p (c b) -> p c b", b=128),
                    in_=x3[0, :, 0:SPLIT0, :])
                nc.scalar.dma_start(
                    out=xt0b[:, :].rearrange("p (c b) -> p c b", b=128),
                    in_=x3[0, :, IMG - SPLIT0:IMG, :])
                xts = [None]
                for s in range(1, n_sg):
                    xt = xin.tile([128, 128 * IMG], mm_dtype, tag=f"xt{s}")
                    eng = nc.sync if s % 2 == 1 else nc.scalar
                    eng.dma_start(out=xt[0:128, :], in_=x_ap[s])
                    xts.append(xt)

                # prime the PE p-state ramp during the xt0a load wait:
                # ~3us of dummy matmuls on already-loaded constants
                prime = fcps.tile([128, 512], F32, tag="ot")
                for _ in range(8):
                    nc.tensor.matmul(prime[0:128, 0:512],
                                     wsb[0:128, 0:128],
                                     wsb[0:128, 0:512],
                                     start=True, stop=True)

                h_all = hbuf.tile([128, n_sg * HSTRIDE], mm_dtype, tag="h")
                hv = h_all[:, :].rearrange("p (s c b) -> p s c b",
                                           s=n_sg, b=128)
                outsb = obuf.tile([128, n_sg * 4 * NCLS], F32, tag="outsb")

                def fc_batch(s0, nb):
                    """FC + bias + transpose for SGs [s0, s0+nb)."""
                    N = nb * 128
                    ot = fcps.tile([128, 512], F32, tag="ot")
                    for c in range(OUT):
                        nc.tensor.matmul(
                            ot[0:128, 0:N],
                            wsb[0:128, 128 * c:128 * c + 128],
                            hv[:, s0:s0 + nb, c, :],
                            start=(c == 0), stop=(c == OUT - 1),
                        )
                    # bias add (per-partition scalar) PSUM -> SBUF
                    osb = obuf.tile([128, 512], F32, tag="osb")
                    nc.vector.tensor_scalar_add(osb[:, 0:N], ot[:, 0:N],
                                                biasv[:, 0:1])
                    if stage < 5:
                        return
                    # DVE 32x32 block transpose: osb[32k+o, 128sl+bq] ->
                    # tt[32k + bq%32, 32*(nb*sl + bq//32) + o]
                    tt = obuf.tile([128, 512], F32, tag="ttbuf")
                    for k in range(4):
                        nc.vector.transpose(tt[32 * k:32 * k + 32, 0:N],
                                            osb[32 * k:32 * k + 32, 0:N])
                    # gather valid o<10 cols of each 32-block into
                    # outsb[:, (s0 + sl)*40 + hi*10 + o]
                    nc.vector.tensor_copy(
                        outsb[:, s0 * 40:(s0 + nb) * 40]
                        .rearrange("p (sl hi o) -> p sl hi o", sl=nb, hi=4),
                        tt[:, 0:N].rearrange("p (sl hi o) -> p sl hi o",
                                             sl=nb, hi=4)[:, :, :, 0:NCLS])
                    if s0 + nb == n_sg:
                        # sample n = (32k + bq%32)*32 + 4s + bq//32:
                        # one DMA, 40B contiguous runs; SP ring is idle now
                        nc.sync.dma_start(
                            out=out_ap[:, :].rearrange(
                                "(p ss hi) o -> p ss hi o", ss=8, hi=4),
                            in_=outsb[:, :].rearrange(
                                "p (ss hi o) -> p ss hi o", ss=8, hi=4))

                for s in range(n_sg):
                    if stage < 2:
                        break
                    # ---- conv: per column-chunk q, 3 accumulated matmuls
                    for q in range(NQ):
                        c0, ncol = _chunk_cols(q)
                        pq = convps.tile([128, 512], F32, tag="pq")
                        for dj in range(3):
                            if s == 0:
                                if q < 3:
                                    xv = xt0a[:, :].rearrange(
                                        "p (c b) -> p c b", b=128)
                                    cl = c0 + dj
                                else:
                                    xv = xt0b[:, :].rearrange(
                                        "p (c b) -> p c b", b=128)
                                    cl = c0 + dj - (IMG - SPLIT0)
                            else:
                                xv = xts[s][:, :].rearrange(
                                    "p (c b) -> p c b", b=128)
                                cl = c0 + dj
                            nc.tensor.matmul(
                                pq[0:128, 0:ncol * 128],
                                msb[0:128, 128 * dj:128 * dj + 128],
                                xv[:, cl:cl + ncol, :],
                                start=(dj == 0), stop=(dj == 2),
                            )
                        # ---- relu PSUM -> SBUF (c-major within SG s) ----
                        if stage < 3:
                            continue
                        hslice = h_all[:, s * HSTRIDE + c0 * 128:
                                       s * HSTRIDE + (c0 + ncol) * 128]
                        if (s * NQ + q) % 2 == 0:
                            nc.vector.tensor_scalar_max(
                                hslice, pq[:, 0:ncol * 128], 0.0)
                        else:
                            nc.scalar.activation(
                                hslice, pq[:, 0:ncol * 128],
                                mybir.ActivationFunctionType.Relu)

                    # ---- FC batches (4,3,1): big batches amortize the 26
                    # weight loads; the last batch is small so the post-PE
                    # drain (bias/transpose/gather/store) tail is short ----
                    if stage >= 4:
                        if s == 3:
                            fc_batch(0, 4)
                        elif s == 6:
                            fc_batch(4, 3)
                        elif s == 7:
                            fc_batch(7, 1)

                if stage < 2:
                    # DMA-only ablation: consume one tile trivially
                    dmy = obuf.tile([128, 8], F32, tag="dmy")
                    nc.vector.tensor_copy(dmy[0:1, 0:8],
                                          xts[1][0:1, 0:16].bitcast(F32))

    nc.compile()
    return nc


def _host_constants(conv_w, W, b):
    """Block-diagonal Toeplitz conv lhsT and block-diagonal FC lhsT."""
    msb = np.zeros((128, 3 * 128), np.float32)
    for dj in range(KW):
        for k in range(4):
            for i in range(OUT):
                for di in range(KW):
                    msb[32 * k + i + di, 128 * dj + 32 * k + i] = conv_w[di, dj]
    wsb = np.zeros((128, OUT * 128), np.float32)
    for c in range(OUT):
        for k in range(4):
            for i in range(OUT):
                wsb[32 * k + i, 128 * c + 32 * k:128 * c + 32 * k + NCLS] = \
                    W[i * OUT + c, :]
    biasv = np.zeros((128, 1), np.float32)
    for k in range(4):
        biasv[32 * k:32 * k + NCLS, 0] = b
    return msb, wsb, biasv


def _marshal_x(x):
    """[B, 784] fp32 -> per-core [N_SG, 128, 3584] fp16 padded layout."""
    # sample n = (32k + bq%32)*32 + 4s + bq//32; bq = 32*hi + bl
    # n axes: [k(4), bl(32), s(8), hi(4)]
    xs = x.reshape(NCORES, 4, 32, N_SG, 4, IMG, IMG)  # core k bl s hi r c
    xs = xs.transpose(0, 3, 1, 5, 6, 4, 2)            # core s k r c hi bl
    xp = np.zeros((NCORES, N_SG, 4, 32, IMG, 4, 32), np.float16)
    xp[:, :, :, 0:IMG] = xs
    return xp.reshape(NCORES, N_SG, 128, 128 * IMG)


def _run(x, conv_w, W, b, trace=False, mm_dtype=F16):
    x = np.ascontiguousarray(np.asarray(x, dtype=np.float32))
    conv_w = np.asarray(conv_w, dtype=np.float32)
    W = np.asarray(W, dtype=np.float32)
    b = np.asarray(b, dtype=np.float32)
    assert x.shape == (B_TOTAL, NPIX), x.shape

    key = ("prog", str(mm_dtype))
    if key not in _CACHE:
        _CACHE[key] = _build_program(mm_dtype)
    nc = _CACHE[key]

    msb, wsb, biasv = _host_constants(conv_w, W, b)
    msb_r, wsb_r = msb.astype(np.float16), wsb.astype(np.float16)
    xm = _marshal_x(x)
    in_maps = []
    for i in range(NCORES):
        in_maps.append({
            "x": xm[i],
            "msb": msb_r, "wsb": wsb_r, "biasv": biasv,
        })
    res = run_bass_kernel_spmd(nc, in_maps, core_ids=list(range(NCORES)),
                               trace=trace)
    out = np.concatenate([res.results[i]["out"] for i in range(NCORES)],
                         axis=0)
    return out, res


def kernel(x, conv_w, W, b):
    out, _ = _run(x, conv_w, W, b, trace=False)
    return out


# revision 33
# speedup vs baseline: 1.8004x; 1.1096x over previous
"""Trainium2 Bass kernel for DigitConvolutionalModel:
    out = relu(conv2d_3x3_valid(x.reshape(B,28,28))) .reshape(B,676) @ W + b

Strategy (pure data parallel over 8 cores, B=32768 -> 4096/core):

Per core, samples are processed in 8 supergroups (SG) of 512 = 128
"quads" of 4 consecutive samples. Host marshals x to a padded
128-partition layout per SG (4 sample-slots x 32 rows, rows 28-31
zero) so every SDMA port participates in the load:
    X[32k + r, (c, bq)] = x[sample(k,bq,s), 28r + c],  r<28
All 8 SG loads are issued up-front on both HWDGE rings (SG0 split in
two column-halves so the first conv matmul starts ~1.8us in).

Conv (cross-correlation) is 3 PSUM-accumulated full-array matmuls per
column-chunk, contracting all 128 rows at once with a block-diagonal
Toeplitz lhsT (M=128, blocks of 26 output rows padded to 32):
    msb[32k + i + di, 128dj + 32k + i] = conv_w[di, dj]
    Y[32k + i, (c, bq)] = sum_dj msb_dj^T @ X[:, (c + dj, bq)]
ReLU copies PSUM->SBUF (alternating DVE/ACT) into fp16
h_all[32k + i, s*3328 + c*128 + bq].

The FC contracts i for all 4 quad-slots at once with a block-diagonal
W lhsT (K=128, M=128), one matmul per output column c over FOUR
supergroups at once (N=512, rhs strided over the s axis of h_all):
    outT[32k + o, (s, bq)] += wsb_c^T @ h_all[:, (s, c, bq)]
so the 26 weight loads are paid twice per pass instead of 8 times.
Bias is added on DVE with a per-partition scalar; DVE 32x32 block
transposes flip outT into sample-major order for a contiguous 40B-run
store (single DMA at the end, on the then-idle SP ring).

All matmul operands are fp16 (values are O(10), well in range); PSUM
accumulates fp32, so the only precision loss is ~2^-11 input rounding.
"""

import sys
import numpy as np

for _p in ("/opt/trn_rl_repo", "/root/.axon_site/_ro/trn_rl_repo"):
    if _p not in sys.path:
        sys.path.insert(0, _p)

import concourse.bass as bass  # noqa: E402,F401
import concourse.tile as tile  # noqa: E402
from concourse import bacc, mybir  # noqa: E402
from concourse.bass_utils import run_bass_kernel_spmd  # noqa: E402

IMG = 28
KW = 3
OUT = 26  # IMG - KW + 1
NPIX = IMG * IMG          # 784
NOUTPIX = OUT * OUT       # 676
NCLS = 10
NCORES = 8
B_TOTAL = 32768
B_CORE = B_TOTAL // NCORES   # 4096
SG = 512                     # samples per supergroup (128 quads of 4)
N_SG = B_CORE // SG          # 8
NQ = 7                       # column chunks: 6x4 + 1x2 = 26 columns
HSTRIDE = OUT * 128          # 3328: per-SG h stride in h_all
F32 = mybir.dt.float32
F16 = mybir.dt.float16

_CACHE = {}


def _chunk_cols(q):
    """(first output column, n columns) of chunk q."""
    return 4 * q, (2 if q == NQ - 1 else 4)


def _build_program(mm_dtype=F16, n_sg=N_SG, hwloop=0, stage=5):
    """Build + compile the per-core Bass program (identical on all cores)."""
    nc = bacc.Bacc("TRN2", target_bir_lowering=False, debug=False,
                   num_devices=NCORES)

    x_d = nc.dram_tensor("x", (N_SG, 128, 128 * IMG), mm_dtype,
                         kind="ExternalInput")
    msb_d = nc.dram_tensor("msb", (128, 3 * 128), mm_dtype,
                           kind="ExternalInput")
    wsb_d = nc.dram_tensor("wsb", (128, OUT * 128), mm_dtype,
                           kind="ExternalInput")
    bias_d = nc.dram_tensor("biasv", (128, 1), F32, kind="ExternalInput")
    out_d = nc.dram_tensor("out", (B_CORE, NCLS), F32, kind="ExternalOutput")

    x_ap = x_d.ap()
    out_ap = out_d.ap()
    NFC = 4                      # supergroups per FC batch
    SPLIT0 = 16                  # SG0 half A columns [0,16), B [12,28)

    with tile.TileContext(nc) as tc:
        with (
            tc.tile_pool(name="consts", bufs=1) as consts,
            tc.tile_pool(name="xin", bufs=1) as xin,
            tc.tile_pool(name="hbuf", bufs=1) as hbuf,
            tc.tile_pool(name="obuf", bufs=2) as obuf,
            tc.tile_pool(name="convps", bufs=7, space="PSUM") as convps,
            tc.tile_pool(name="fcps", bufs=1, space="PSUM") as fcps,
        ):
            msb = consts.tile([128, 3 * 128], mm_dtype)
            wsb = consts.tile([128, OUT * 128], mm_dtype)
            biasv = consts.tile([128, 1], F32)
            # msb gates the first conv: tiny, first on the SP ring.  wsb
            # (852KB, needed ~20us in) + biasv go via SWDGE so neither
            # HWDGE ring delays the x loads in a cold single pass.
            primec = consts.tile([128, 640], mm_dtype)
            nc.vector.memset(primec[:, :], 0.0)
            nc.sync.dma_start(out=msb[:, :], in_=msb_d.ap())
            nc.gpsimd.dma_start(out=wsb[:, :], in_=wsb_d.ap())
            nc.gpsimd.dma_start(out=biasv[:, :], in_=bias_d.ap())

            import contextlib
            loop_cm = (tc.For_i(0, hwloop, 1) if hwloop
                       else contextlib.nullcontext())
            with loop_cm:
                # ---- all SG loads up-front, spread over both HWDGE rings;
                # SG0 in two column-halves so conv can start early ----
                xt0a = xin.tile([128, SPLIT0 * 128], mm_dtype, tag="xt0a")
                xt0b = xin.tile([128, SPLIT0 * 128], mm_dtype, tag="xt0b")
                x3 = x_ap.rearrange("s p (c b) -> s p c b", b=128)
                nc.sync.dma_start(
                    out=xt0a[:, :].rearrange("p (c b) -> p c b", b=128),
                    in_=x3[0, :, 0:SPLIT0, :])
                nc.scalar.dma_start(
                    out=xt0b[:, :].rearrange("p (c b) -> p c b", b=128),
                    in_=x3[0, :, IMG - SPLIT0:IMG, :])
                xts = [None]
                for s in range(1, n_sg):
                    xt = xin.tile([128, 128 * IMG], mm_dtype, tag=f"xt{s}")
                    eng = nc.sync if s % 2 == 1 else nc.scalar
                    eng.dma_start(out=xt[0:128, :], in_=x_ap[s])
                    xts.append(xt)

                # prime the PE p-state ramp during the xt0a load wait:
                # ~3us of dummy matmuls on already-loaded constants
                prime = fcps.tile([128, 512], F32, tag="ot")
                for _ in range(8):
                    nc.tensor.matmul(prime[0:128, 0:512],
                                     primec[0:128, 0:128],
                                     primec[0:128, 128:640],
                                     start=True, stop=True)

                h_all = hbuf.tile([128, n_sg * HSTRIDE], mm_dtype, tag="h")
                hv = h_all[:, :].rearrange("p (s c b) -> p s c b",
                                           s=n_sg, b=128)
                outsb = obuf.tile([128, n_sg * 4 * NCLS], F32, tag="outsb")

                def fc_batch(s0, nb):
                    """FC + bias + transpose for SGs [s0, s0+nb)."""
                    N = nb * 128
                    ot = fcps.tile([128, 512], F32, tag="ot")
                    for c in range(OUT):
                        nc.tensor.matmul(
                            ot[0:128, 0:N],
                            wsb[0:128, 128 * c:128 * c + 128],
                            hv[:, s0:s0 + nb, c, :],
                            start=(c == 0), stop=(c == OUT - 1),
                        )
                    # bias add (per-partition scalar) PSUM -> SBUF
                    osb = obuf.tile([128, 512], F32, tag="osb")
                    nc.vector.tensor_scalar_add(osb[:, 0:N], ot[:, 0:N],
                                                biasv[:, 0:1])
                    if stage < 5:
                        return
                    # DVE 32x32 block transpose: osb[32k+o, 128sl+bq] ->
                    # tt[32k + bq%32, 32*(nb*sl + bq//32) + o]
                    tt = obuf.tile([128, 512], F32, tag="ttbuf")
                    for k in range(4):
                        nc.vector.transpose(tt[32 * k:32 * k + 32, 0:N],
                                            osb[32 * k:32 * k + 32, 0:N])
                    # gather valid o<10 cols of each 32-block into
                    # outsb[:, (s0 + sl)*40 + hi*10 + o]
                    nc.vector.tensor_copy(
                        outsb[:, s0 * 40:(s0 + nb) * 40]
                        .rearrange("p (sl hi o) -> p sl hi o", sl=nb, hi=4),
                        tt[:, 0:N].rearrange("p (sl hi o) -> p sl hi o",
                                             sl=nb, hi=4)[:, :, :, 0:NCLS])
                    if s0 + nb == n_sg:
                        # sample n = (32k + bq%32)*32 + 4s + bq//32:
                        # one DMA, 40B contiguous runs; SP ring is idle now
                        nc.sync.dma_start(
                            out=out_ap[:, :].rearrange(
                                "(p ss hi) o -> p ss hi o", ss=8, hi=4),
                            in_=outsb[:, :].rearrange(
                                "p (ss hi o) -> p ss hi o", ss=8, hi=4))

                for s in range(n_sg):
                    if stage < 2:
                        break
                    # ---- conv: per column-chunk q, 3 accumulated matmuls
                    for q in range(NQ):
                        c0, ncol = _chunk_cols(q)
                        pq = convps.tile([128, 512], F32, tag="pq")
                        for dj in range(3):
                            if s == 0:
                                if q < 3:
                                    xv = xt0a[:, :].rearrange(
                                        "p (c b) -> p c b", b=128)
                                    cl = c0 + dj
                                else:
                                    xv = xt0b[:, :].rearrange(
                                        "p (c b) -> p c b", b=128)
                                    cl = c0 + dj - (IMG - SPLIT0)
                            else:
                                xv = xts[s][:, :].rearrange(
                                    "p (c b) -> p c b", b=128)
                                cl = c0 + dj
                            nc.tensor.matmul(
                                pq[0:128, 0:ncol * 128],
                                msb[0:128, 128 * dj:128 * dj + 128],
                                xv[:, cl:cl + ncol, :],
                                start=(dj == 0), stop=(dj == 2),
                            )
                        # ---- relu PSUM -> SBUF (c-major within SG s) ----
                        if stage < 3:
                            continue
                        hslice = h_all[:, s * HSTRIDE + c0 * 128:
                                       s * HSTRIDE + (c0 + ncol) * 128]
                        if (s * NQ + q) % 2 == 0:
                            nc.vector.tensor_scalar_max(
                                hslice, pq[:, 0:ncol * 128], 0.0)
                        else:
                            nc.scalar.activation(
                                hslice, pq[:, 0:ncol * 128],
                                mybir.ActivationFunctionType.Relu)

                    # ---- FC batches (4,3,1): big batches amortize the 26
                    # weight loads; the last batch is small so the post-PE
                    # drain (bias/transpose/gather/store) tail is short ----
                    if stage >= 4:
                        if s == 3:
                            fc_batch(0, 4)
                        elif s == 6:
                            fc_batch(4, 3)
                        elif s == 7:
                            fc_batch(7, 1)

                if stage < 2:
                    # DMA-only ablation: consume one tile trivially
                    dmy = obuf.tile([128, 8], F32, tag="dmy")
                    nc.vector.tensor_copy(dmy[0:1, 0:8],
                                          xts[1][0:1, 0:16].bitcast(F32))

    nc.compile()
    return nc


def _host_constants(conv_w, W, b):
    """Block-diagonal Toeplitz conv lhsT and block-diagonal FC lhsT."""
    msb = np.zeros((128, 3 * 128), np.float32)
    for dj in range(KW):
        for k in range(4):
            for i in range(OUT):
                for di in range(KW):
                    msb[32 * k + i + di, 128 * dj + 32 * k + i] = conv_w[di, dj]
    wsb = np.zeros((128, OUT * 128), np.float32)
    for c in range(OUT):
        for k in range(4):
            for i in range(OUT):
                wsb[32 * k + i, 128 * c + 32 * k:128 * c + 32 * k + NCLS] = \
                    W[i * OUT + c, :]
    biasv = np.zeros((128, 1), np.float32)
    for k in range(4):
        biasv[32 * k:32 * k + NCLS, 0] = b
    return msb, wsb, biasv


def _marshal_x(x):
    """[B, 784] fp32 -> per-core [N_SG, 128, 3584] fp16 padded layout."""
    # sample n = (32k + bq%32)*32 + 4s + bq//32; bq = 32*hi + bl
    # n axes: [k(4), bl(32), s(8), hi(4)]
    xs = x.reshape(NCORES, 4, 32, N_SG, 4, IMG, IMG)  # core k bl s hi r c
    xs = xs.transpose(0, 3, 1, 5, 6, 4, 2)            # core s k r c hi bl
    xp = np.zeros((NCORES, N_SG, 4, 32, IMG, 4, 32), np.float16)
    xp[:, :, :, 0:IMG] = xs
    return xp.reshape(NCORES, N_SG, 128, 128 * IMG)


def _run(x, conv_w, W, b, trace=False, mm_dtype=F16):
    x = np.ascontiguousarray(np.asarray(x, dtype=np.float32))
    conv_w = np.asarray(conv_w, dtype=np.float32)
    W = np.asarray(W, dtype=np.float32)
    b = np.asarray(b, dtype=np.float32)
    assert x.shape == (B_TOTAL, NPIX), x.shape

    key = ("prog", str(mm_dtype))
    if key not in _CACHE:
        _CACHE[key] = _build_program(mm_dtype)
    nc = _CACHE[key]

    msb, wsb, biasv = _host_constants(conv_w, W, b)
    msb_r, wsb_r = msb.astype(np.float16), wsb.astype(np.float16)
    xm = _marshal_x(x)
    in_maps = []
    for i in range(NCORES):
        in_maps.append({
            "x": xm[i],
            "msb": msb_r, "wsb": wsb_r, "biasv": biasv,
        })
    res = run_bass_kernel_spmd(nc, in_maps, core_ids=list(range(NCORES)),
                               trace=trace)
    out = np.concatenate([res.results[i]["out"] for i in range(NCORES)],
                         axis=0)
    return out, res


def kernel(x, conv_w, W, b):
    out, _ = _run(x, conv_w, W, b, trace=False)
    return out
